# revision 10
# baseline (speedup 1.0000x reference)
"""CrossAttention (reverse-weight) Trainium2 kernel, v2.

Data-parallel over batch B=8 across 8 NeuronCores (one batch per core).

Math (per batch), identical to v1:
    q = x1 @ Wq, k = x2 @ Wk, v = x2 @ Wv   (bq zero; bk softmax-invariant)
    E = exp(q @ k.T / 8)   (no max-shift needed: |scores| <~ 2)
    attn = (colsum(v) - (E@v)/rowsum(E)) / (S-1)
    out = layernorm(attn) * gamma + beta, with the 1/(S-1) folded into eps.
    colsum(v) + (S-1)*bv computed host-side in float64 (vsumB).

v2 changes vs v1 (which ran ~151 us):
  - All matmul operands bf16 (halves DMA, enables FWL weight loads, keeps
    fp32 PSUM accumulate). exp output et is bf16.
  - No PE transposes at all: v-tiles and the attnT epilogue are transposed
    with the DMA xbar (16-bit SBUF->SBUF), freeing PSUM and PE time.
  - Single 8-bank PSUM pool: sc[128,1024]x2 (banks 0-3), at[65,1024]x1
    (4-5), proj[128,1024]x1 (6-7, rotating kv-h0 -> q-h0 -> kv-h1 -> q-h1
    in DMA arrival order x2h0, x1h0, x2h1, x1h1).
  - Stage 2 is h-outer (q-half outer, s-tile inner) so half 0's epilogue
    overlaps half 1's score/exp/attn pipeline; ACT exp in [128,1024]
    instructions is the expected steady-state bottleneck (~1.15 us/pair).
"""

import numpy as np

import concourse.bacc as bacc
import concourse.tile as tile
from concourse import mybir
from concourse.bass_utils import run_bass_kernel_spmd

F32 = mybir.dt.float32
BF16 = mybir.dt.bfloat16
AF = mybir.ActivationFunctionType
ALU = mybir.AluOpType

B, S, DM, DK, DV = 8, 2048, 768, 64, 64
NT = S // 128           # 16 s-tiles / q-tiles
NTH = NT // 2           # 8 tiles per q-half
NC_CHUNKS = DM // 128   # 6 contraction chunks
EPS_EFF = 1e-5 * float(S - 1) * float(S - 1)  # 41.90209
N_CORES = 8


def build_program():
    nc = bacc.Bacc(None)

    x1t = nc.declare_dram_parameter("x1t", [DM, S], BF16, isOutput=False)
    x2t = nc.declare_dram_parameter("x2t", [DM, S], BF16, isOutput=False)
    wq = nc.declare_dram_parameter("wq", [DM, DK], BF16, isOutput=False)
    wkv = nc.declare_dram_parameter("wkv", [DM, 2 * DK], BF16, isOutput=False)
    vsb = nc.declare_dram_parameter("vsb", [DV], F32, isOutput=False)
    out = nc.declare_dram_parameter("out", [S, DV], F32, isOutput=True)

    with tile.TileContext(nc) as tc:
        _emit(nc, tc, x1t, x2t, wq, wkv, vsb, out)
    nc.finalize()
    return nc


def _emit(nc, tc, x1t, x2t, wq, wkv, vsb, out):
    from contextlib import ExitStack

    ctx = ExitStack()
    with ctx:
        singles = ctx.enter_context(tc.tile_pool(name="singles", bufs=1))
        xpool = ctx.enter_context(tc.tile_pool(name="xpool", bufs=1))
        sbuf = ctx.enter_context(tc.tile_pool(name="sbuf", bufs=1))
        et_pool = ctx.enter_context(tc.tile_pool(name="et_pool", bufs=3))
        ep_pool = ctx.enter_context(tc.tile_pool(name="ep_pool", bufs=2))

        from concourse.masks import make_identity

        # ---- constants / weights ----
        ident = singles.tile([128, 128], BF16)
        make_identity(nc, ident)
        eps_sb = singles.tile([128, 1], F32)
        nc.vector.memset(eps_sb, EPS_EFF)
        wq_sb = singles.tile([128, NC_CHUNKS, DK], BF16)
        nc.scalar.dma_start(out=wq_sb, in_=wq.rearrange("(c p) m -> p c m", p=128))
        wkv_sb = singles.tile([128, NC_CHUNKS, 2 * DK], BF16)
        nc.scalar.dma_start(out=wkv_sb, in_=wkv.rearrange("(c p) m -> p c m", p=128))
        # vsumB = colsum(v) + (S-1)*bv, host-computed, broadcast to all rows
        vsumB = singles.tile([128, DV], F32)
        nc.scalar.dma_start(out=vsumB, in_=vsb.ap().partition_broadcast(128))

        # v tiles [128, NT, 66]: cols 0:64 = v rows (s-major), col 64 = -1.0
        v_sb = sbuf.tile([128, NT, DK + 2], BF16)
        m1_sb = singles.tile([128, NT], BF16)
        nc.vector.memset(m1_sb, -1.0)
        nc.vector.tensor_copy(v_sb[:, :, DK], m1_sb)

        # ---- x piece DMAs [128, 1024] bf16; arrival order drives the proj
        # rotation on psum banks 6-7: kv-h0, q-h0, kv-h1, q-h1 ----
        x1_sb = [[None] * 2 for _ in range(NC_CHUNKS)]
        x2_sb = [[None] * 2 for _ in range(NC_CHUNKS)]

        def load_piece(dst, src, c, h, tag, eng):
            t = xpool.tile([128, 1024], BF16, tag=f"{tag}_{c}_{h}",
                           name=f"{tag}_{c}_{h}")
            eng.dma_start(
                out=t, in_=src[c * 128:(c + 1) * 128, h * 1024:(h + 1) * 1024]
            )
            dst[c][h] = t

        for c in range(NC_CHUNKS):
            load_piece(x2_sb, x2t, c, 0, "x2", nc.sync)
        for c in range(NC_CHUNKS):
            load_piece(x1_sb, x1t, c, 0, "x1", nc.gpsimd)
        for c in range(NC_CHUNKS):
            load_piece(x2_sb, x2t, c, 1, "x2", nc.sync)
        for c in range(NC_CHUNKS):
            load_piece(x1_sb, x1t, c, 1, "x1", nc.gpsimd)

        qT_sb = sbuf.tile([64, S], BF16)
        kv_sb = sbuf.tile([128, S], BF16)

        # ---- the single PSUM pool: 4+4+4+4 KB/partition = all 16 KB ----
        psum = ctx.enter_context(tc.tile_pool(name="psum", bufs=1, space="PSUM"))

        def project(dst_sb, w_sb, x_sb, h, nrow, tag):
            p = psum.tile([128, 1024], F32, tag="proj", name=f"proj_{tag}")
            for c in range(NC_CHUNKS):
                for blk in range(2):
                    lo = blk * 512
                    nc.tensor.matmul(
                        p[0:nrow, lo:lo + 512],
                        w_sb[:, c, :],
                        x_sb[c][h][:, lo:lo + 512],
                        start=(c == 0),
                        stop=(c == NC_CHUNKS - 1),
                    )
            nc.vector.tensor_copy(
                dst_sb[:, h * 1024:(h + 1) * 1024], p[0:nrow, :]
            )

        # ---- stage 2 + epilogue state ----
        out_sb = sbuf.tile([128, NT, DV], F32)
        mv_all = sbuf.tile([128, 2, NTH, 2], F32)   # [., half, tile, (mean,var)]
        std_all = sbuf.tile([128, 2, NTH], F32)
        rs_all = sbuf.tile([128, 2, NTH], F32)
        at_tiles = [None, None]
        ep_state = [None, None]

        def vtrans(h):
            for t in range(NTH):
                i = h * NTH + t
                trp = psum.tile([128, DK], BF16, tag="proj", name=f"vtr{i}")
                nc.tensor.transpose(
                    trp,
                    kv_sb[64:128, i * 128:(i + 1) * 128],
                    ident[64:128, 64:128],
                )
                nc.vector.tensor_copy(v_sb[:, i, 0:DK], trp)

        def pair(h, i):
            kt_i = kv_sb[0:64, i * 128:(i + 1) * 128]
            sc = psum.tile([128, 1024], F32, tag="sc", bufs=2, name="sc")
            for blk in range(2):
                qlo = h * 1024 + blk * 512
                nc.tensor.matmul(
                    sc[:, blk * 512:(blk + 1) * 512],
                    kt_i,
                    qT_sb[:, qlo:qlo + 512],
                    start=True,
                    stop=True,
                )
            et = et_pool.tile([128, 1024], BF16, tag="et", name="et")
            nc.scalar.activation(et, sc, AF.Exp, scale=0.125)
            for blk in range(2):
                nc.tensor.matmul(
                    at_tiles[h][:, blk * 512:(blk + 1) * 512],
                    v_sb[:, i, 0:DV + 1],
                    et[:, blk * 512:(blk + 1) * 512],
                    start=(i == 0),
                    stop=(i == NT - 1),
                )

        def ep_begin(h):
            # at rows: 0:64 = EV (v-major), row 64 = -rowsum. bf16 roundoff on
            # EV is ~1e-6 of the final output (EV/rowsum << layernorm scale).
            at_sbh = ep_pool.tile([DV + 1, 1024], BF16, tag="at_sb",
                                  name=f"atsb{h}")
            nc.vector.tensor_copy(at_sbh, at_tiles[h])
            a_all = ep_pool.tile([128, NTH, DV + 2], F32, tag="a_all",
                                 name=f"a_all{h}")
            ep_state[h] = (at_sbh, a_all)

        def ep_transpose(h, t):
            at_sbh, a_all = ep_state[h]
            trp = psum.tile([128, DV + 1], BF16, tag="proj",
                            name=f"eptr{h}_{t}")
            nc.tensor.transpose(
                trp,
                at_sbh[:, t * 128:(t + 1) * 128],
                ident[0:DV + 1, 0:DV + 1],
            )
            nc.vector.tensor_copy(a_all[:, t, 0:DV + 1], trp)

        def ep_stats(h):
            _, a_all = ep_state[h]
            # rneg = -1/rowsum for all 8 tiles at once (col 64 = -rowsum)
            rneg = ep_pool.tile([128, NTH], F32, tag="rneg", name=f"rneg{h}")
            nc.vector.reciprocal(rneg, a_all[:, :, DV])
            tts = []
            for t in range(NTH):
                tt = ep_pool.tile([128, DV], F32, tag=f"tt{t}", name=f"tt{t}")
                tts.append(tt)
                # t = EV * (-1/rowsum) + vsumB
                nc.vector.scalar_tensor_tensor(
                    out=tt,
                    in0=a_all[:, t, 0:DV],
                    scalar=rneg[:, t:t + 1],
                    in1=vsumB,
                    op0=ALU.mult,
                    op1=ALU.add,
                )
                stats = ep_pool.tile([128, 6], F32, tag="stats", name="stats")
                nc.vector.bn_stats(out=stats, in_=tt)
                nc.vector.bn_aggr(out=mv_all[:, h, t, :], in_=stats)
            # batched std = sqrt(var + eps) and rs = 1/std for the 8 tiles
            nc.scalar.activation(
                std_all[:, h, :], mv_all[:, h, :, 1], AF.Sqrt,
                bias=eps_sb, scale=1.0,
            )
            nc.vector.reciprocal(rs_all[:, h, :], std_all[:, h, :])
            for t in range(NTH):
                gt = t + h * NTH
                nc.vector.tensor_scalar(
                    out=out_sb[:, gt, :],
                    in0=tts[t],
                    scalar1=mv_all[:, h, t, 0:1],
                    scalar2=rs_all[:, h, t:t + 1],
                    op0=ALU.subtract,
                    op1=ALU.mult,
                )
            nc.sync.dma_start(
                out=out.rearrange("(t p) j -> p t j", p=128)[
                    :, h * NTH:(h + 1) * NTH, :
                ],
                in_=out_sb[:, h * NTH:(h + 1) * NTH, :],
            )

        # ---- interleaved emission in data-arrival order: the PE stream must
        # never make early work wait behind matmuls whose DMA lands later ----
        project(kv_sb, wkv_sb, x2_sb, 0, 128, "kv0")
        vtrans(0)
        project(qT_sb, wq_sb, x1_sb, 0, 64, "q0")
        at_tiles[0] = psum.tile([DV + 1, 1024], F32, tag="at", name="at0")
        for i in range(NTH):          # h0 pairs on s-tiles 0..7
            pair(0, i)
        project(kv_sb, wkv_sb, x2_sb, 1, 128, "kv1")
        vtrans(1)
        for i in range(NTH, NT):      # h0 pairs on s-tiles 8..15
            pair(0, i)
        project(qT_sb, wq_sb, x1_sb, 1, 64, "q1")
        ep_begin(0)
        at_tiles[1] = psum.tile([DV + 1, 1024], F32, tag="at", name="at1")
        for i in range(NT):           # h1 pairs; h0 epilogue rides PE slack
            pair(1, i)
            if i < NTH:
                ep_transpose(0, i)
        ep_stats(0)
        ep_begin(1)
        for t in range(NTH):
            ep_transpose(1, t)
        ep_stats(1)

_NC_CACHE = None


def _get_nc():
    global _NC_CACHE
    if _NC_CACHE is None:
        _NC_CACHE = build_program()
    return _NC_CACHE


def make_in_maps(x_1, x_2, Wq, Wk, Wv, bv):
    import ml_dtypes

    bf16 = ml_dtypes.bfloat16
    x1t = np.ascontiguousarray(x_1.transpose(0, 2, 1)).astype(bf16)  # [B,DM,S]
    x2t = np.ascontiguousarray(x_2.transpose(0, 2, 1)).astype(bf16)
    wkv = np.ascontiguousarray(np.concatenate([Wk, Wv], axis=1)).astype(bf16)
    wqb = Wq.astype(bf16)
    # colsum(v) + (S-1)*bv in float64 for exactness (it dominates t and must
    # not inherit device rounding)
    vsb = (
        x_2.astype(np.float64).sum(axis=1) @ Wv.astype(np.float64)
        + np.float64(S - 1) * bv.astype(np.float64)
    ).astype(np.float32)  # [B, DV]
    return [
        {"x1t": x1t[b], "x2t": x2t[b], "wq": wqb, "wkv": wkv, "vsb": vsb[b]}
        for b in range(B)
    ]


def kernel(**inputs):
    x_1 = np.asarray(inputs["x_1"], np.float32)
    x_2 = np.asarray(inputs["x_2"], np.float32)
    Wq = np.asarray(inputs["Wq"], np.float32)
    Wk = np.asarray(inputs["Wk"], np.float32)
    Wv = np.asarray(inputs["Wv"], np.float32)
    bv = np.asarray(inputs["bv"], np.float32)
    gamma = np.asarray(inputs["gamma"], np.float32)
    beta = np.asarray(inputs["beta"], np.float32)

    nc = _get_nc()
    in_maps = make_in_maps(x_1, x_2, Wq, Wk, Wv, bv)
    res = run_bass_kernel_spmd(nc, in_maps, list(range(N_CORES)))
    outs = np.stack([res.results[b]["out"] for b in range(B)], axis=0)
    return (outs * gamma + beta).astype(np.float32)


# revision 11
# speedup vs baseline: 1.0803x; 1.0803x over previous
"""CrossAttention (reverse-weight) Trainium2 kernel, v2.

Data-parallel over batch B=8 across 8 NeuronCores (one batch per core).

Math (per batch), identical to v1:
    q = x1 @ Wq, k = x2 @ Wk, v = x2 @ Wv   (bq zero; bk softmax-invariant)
    E = exp(q @ k.T / 8)   (no max-shift needed: |scores| <~ 2)
    attn = (colsum(v) - (E@v)/rowsum(E)) / (S-1)
    out = layernorm(attn) * gamma + beta, with the 1/(S-1) folded into eps.
    colsum(v) + (S-1)*bv computed host-side in float64 (vsumB).

v2 changes vs v1 (which ran ~151 us):
  - All matmul operands bf16 (halves DMA, enables FWL weight loads, keeps
    fp32 PSUM accumulate). exp output et is bf16.
  - No PE transposes at all: v-tiles and the attnT epilogue are transposed
    with the DMA xbar (16-bit SBUF->SBUF), freeing PSUM and PE time.
  - Single 8-bank PSUM pool: sc[128,1024]x2 (banks 0-3), at[65,1024]x1
    (4-5), proj[128,1024]x1 (6-7, rotating kv-h0 -> q-h0 -> kv-h1 -> q-h1
    in DMA arrival order x2h0, x1h0, x2h1, x1h1).
  - Stage 2 is h-outer (q-half outer, s-tile inner) so half 0's epilogue
    overlaps half 1's score/exp/attn pipeline; ACT exp in [128,1024]
    instructions is the expected steady-state bottleneck (~1.15 us/pair).
"""

import numpy as np

import concourse.bacc as bacc
import concourse.tile as tile
from concourse import mybir
from concourse.bass_utils import run_bass_kernel_spmd

F32 = mybir.dt.float32
BF16 = mybir.dt.bfloat16
AF = mybir.ActivationFunctionType
ALU = mybir.AluOpType

B, S, DM, DK, DV = 8, 2048, 768, 64, 64
NT = S // 128           # 16 s-tiles / q-tiles
NTH = NT // 2           # 8 tiles per q-half
NC_CHUNKS = DM // 128   # 6 contraction chunks
EPS_EFF = 1e-5 * float(S - 1) * float(S - 1)  # 41.90209
N_CORES = 8


def build_program():
    nc = bacc.Bacc(None)

    x1t = nc.declare_dram_parameter("x1t", [DM, S], BF16, isOutput=False)
    x2t = nc.declare_dram_parameter("x2t", [DM, S], BF16, isOutput=False)
    wq = nc.declare_dram_parameter("wq", [DM, DK], BF16, isOutput=False)
    wkv = nc.declare_dram_parameter("wkv", [DM, 2 * DK], BF16, isOutput=False)
    vsb = nc.declare_dram_parameter("vsb", [DV], F32, isOutput=False)
    out = nc.declare_dram_parameter("out", [S, DV], F32, isOutput=True)

    with tile.TileContext(nc) as tc:
        _emit(nc, tc, x1t, x2t, wq, wkv, vsb, out)
    nc.finalize()
    return nc


def _emit(nc, tc, x1t, x2t, wq, wkv, vsb, out):
    from contextlib import ExitStack

    ctx = ExitStack()
    with ctx:
        singles = ctx.enter_context(tc.tile_pool(name="singles", bufs=1))
        xpool = ctx.enter_context(tc.tile_pool(name="xpool", bufs=1))
        sbuf = ctx.enter_context(tc.tile_pool(name="sbuf", bufs=1))
        et_pool = ctx.enter_context(tc.tile_pool(name="et_pool", bufs=3))
        ep_pool = ctx.enter_context(tc.tile_pool(name="ep_pool", bufs=2))

        from concourse.masks import make_identity

        # ---- constants / weights ----
        ident = singles.tile([128, 128], BF16)
        make_identity(nc, ident)
        eps_sb = singles.tile([128, 1], F32)
        nc.vector.memset(eps_sb, EPS_EFF)
        wq_sb = singles.tile([128, NC_CHUNKS, DK], BF16)
        nc.sync.dma_start(out=wq_sb, in_=wq.rearrange("(c p) m -> p c m", p=128))
        wkv_sb = singles.tile([128, NC_CHUNKS, 2 * DK], BF16)
        nc.sync.dma_start(out=wkv_sb, in_=wkv.rearrange("(c p) m -> p c m", p=128))
        # vsumB = colsum(v) + (S-1)*bv, host-computed, broadcast to all rows
        vsumB = singles.tile([128, DV], F32)
        nc.sync.dma_start(out=vsumB, in_=vsb.ap().partition_broadcast(128))

        # v tiles [128, NT, 66]: cols 0:64 = v rows (s-major), col 64 = -1.0
        v_sb = sbuf.tile([128, NT, DK + 2], BF16)
        m1_sb = singles.tile([128, NT], BF16)
        nc.vector.memset(m1_sb, -1.0)
        nc.vector.tensor_copy(v_sb[:, :, DK], m1_sb)

        # ---- x piece DMAs [128, 1024] bf16; arrival order drives the proj
        # rotation on psum banks 6-7: kv-h0, q-h0, kv-h1, q-h1 ----
        x1_sb = [[None] * 2 for _ in range(NC_CHUNKS)]
        x2_sb = [[None] * 2 for _ in range(NC_CHUNKS)]

        def load_piece(dst, src, c, h, tag, eng):
            t = xpool.tile([128, 1024], BF16, tag=f"{tag}_{c}_{h}",
                           name=f"{tag}_{c}_{h}")
            eng.dma_start(
                out=t, in_=src[c * 128:(c + 1) * 128, h * 1024:(h + 1) * 1024]
            )
            dst[c][h] = t

        for c in range(NC_CHUNKS):
            load_piece(x2_sb, x2t, c, 0, "x2", nc.sync)
        for c in range(NC_CHUNKS):
            load_piece(x1_sb, x1t, c, 0, "x1", nc.sync)
        for c in range(NC_CHUNKS):
            load_piece(x2_sb, x2t, c, 1, "x2", nc.sync)
        for c in range(NC_CHUNKS):
            load_piece(x1_sb, x1t, c, 1, "x1", nc.sync)

        qT_sb = sbuf.tile([64, S], BF16)
        kv_sb = sbuf.tile([128, S], BF16)

        # ---- the single PSUM pool: 4+4+4+4 KB/partition = all 16 KB ----
        psum = ctx.enter_context(tc.tile_pool(name="psum", bufs=1, space="PSUM"))

        def project(dst_sb, w_sb, x_sb, h, nrow, tag):
            p = psum.tile([128, 1024], F32, tag="proj", name=f"proj_{tag}")
            for c in range(NC_CHUNKS):
                for blk in range(2):
                    lo = blk * 512
                    nc.tensor.matmul(
                        p[0:nrow, lo:lo + 512],
                        w_sb[:, c, :],
                        x_sb[c][h][:, lo:lo + 512],
                        start=(c == 0),
                        stop=(c == NC_CHUNKS - 1),
                    )
            nc.vector.tensor_copy(
                dst_sb[:, h * 1024:(h + 1) * 1024], p[0:nrow, :]
            )

        # ---- stage 2 + epilogue state ----
        out_sb = sbuf.tile([128, NT, DV], F32)
        mv_all = sbuf.tile([128, 2, NTH, 2], F32)   # [., half, tile, (mean,var)]
        std_all = sbuf.tile([128, 2, NTH], F32)
        rs_all = sbuf.tile([128, 2, NTH], F32)
        at_tiles = [None, None]
        ep_state = [None, None]

        def vtrans(h):
            for t in range(NTH):
                i = h * NTH + t
                trp = psum.tile([128, DK], BF16, tag="proj", name=f"vtr{i}")
                nc.tensor.transpose(
                    trp,
                    kv_sb[64:128, i * 128:(i + 1) * 128],
                    ident[64:128, 64:128],
                )
                nc.vector.tensor_copy(v_sb[:, i, 0:DK], trp)

        def pair(h, i):
            kt_i = kv_sb[0:64, i * 128:(i + 1) * 128]
            sc = psum.tile([128, 1024], F32, tag="sc", bufs=2, name="sc")
            for blk in range(2):
                qlo = h * 1024 + blk * 512
                nc.tensor.matmul(
                    sc[:, blk * 512:(blk + 1) * 512],
                    kt_i,
                    qT_sb[:, qlo:qlo + 512],
                    start=True,
                    stop=True,
                )
            et = et_pool.tile([128, 1024], BF16, tag="et", name="et")
            nc.scalar.activation(et, sc, AF.Exp, scale=0.125)
            for blk in range(2):
                nc.tensor.matmul(
                    at_tiles[h][:, blk * 512:(blk + 1) * 512],
                    v_sb[:, i, 0:DV + 1],
                    et[:, blk * 512:(blk + 1) * 512],
                    start=(i == 0),
                    stop=(i == NT - 1),
                )

        def ep_begin(h):
            # at rows: 0:64 = EV (v-major), row 64 = -rowsum. bf16 roundoff on
            # EV is ~1e-6 of the final output (EV/rowsum << layernorm scale).
            at_sbh = ep_pool.tile([DV + 1, 1024], BF16, tag="at_sb",
                                  name=f"atsb{h}")
            nc.vector.tensor_copy(at_sbh, at_tiles[h])
            a_all = ep_pool.tile([128, NTH, DV + 2], F32, tag="a_all",
                                 name=f"a_all{h}")
            ep_state[h] = (at_sbh, a_all)

        def ep_transpose(h, t):
            at_sbh, a_all = ep_state[h]
            trp = psum.tile([128, DV + 1], BF16, tag="proj",
                            name=f"eptr{h}_{t}")
            nc.tensor.transpose(
                trp,
                at_sbh[:, t * 128:(t + 1) * 128],
                ident[0:DV + 1, 0:DV + 1],
            )
            nc.vector.tensor_copy(a_all[:, t, 0:DV + 1], trp)

        def ep_stats(h):
            _, a_all = ep_state[h]
            # rneg = -1/rowsum for all 8 tiles at once (col 64 = -rowsum)
            rneg = ep_pool.tile([128, NTH], F32, tag="rneg", name=f"rneg{h}")
            nc.vector.reciprocal(rneg, a_all[:, :, DV])
            tts = []
            for t in range(NTH):
                tt = ep_pool.tile([128, DV], F32, tag=f"tt{t}", name=f"tt{t}")
                tts.append(tt)
                # t = EV * (-1/rowsum) + vsumB
                nc.vector.scalar_tensor_tensor(
                    out=tt,
                    in0=a_all[:, t, 0:DV],
                    scalar=rneg[:, t:t + 1],
                    in1=vsumB,
                    op0=ALU.mult,
                    op1=ALU.add,
                )
                stats = ep_pool.tile([128, 6], F32, tag="stats", name="stats")
                nc.vector.bn_stats(out=stats, in_=tt)
                nc.vector.bn_aggr(out=mv_all[:, h, t, :], in_=stats)
            # batched std = sqrt(var + eps) and rs = 1/std for the 8 tiles
            nc.scalar.activation(
                std_all[:, h, :], mv_all[:, h, :, 1], AF.Sqrt,
                bias=eps_sb, scale=1.0,
            )
            nc.vector.reciprocal(rs_all[:, h, :], std_all[:, h, :])
            for t in range(NTH):
                gt = t + h * NTH
                nc.vector.tensor_scalar(
                    out=out_sb[:, gt, :],
                    in0=tts[t],
                    scalar1=mv_all[:, h, t, 0:1],
                    scalar2=rs_all[:, h, t:t + 1],
                    op0=ALU.subtract,
                    op1=ALU.mult,
                )
            nc.sync.dma_start(
                out=out.rearrange("(t p) j -> p t j", p=128)[
                    :, h * NTH:(h + 1) * NTH, :
                ],
                in_=out_sb[:, h * NTH:(h + 1) * NTH, :],
            )

        # ---- interleaved emission in data-arrival order: the PE stream must
        # never make early work wait behind matmuls whose DMA lands later ----
        project(kv_sb, wkv_sb, x2_sb, 0, 128, "kv0")
        vtrans(0)
        project(qT_sb, wq_sb, x1_sb, 0, 64, "q0")
        at_tiles[0] = psum.tile([DV + 1, 1024], F32, tag="at", name="at0")
        for i in range(NTH):          # h0 pairs on s-tiles 0..7
            pair(0, i)
        project(kv_sb, wkv_sb, x2_sb, 1, 128, "kv1")
        vtrans(1)
        for i in range(NTH, NT):      # h0 pairs on s-tiles 8..15
            pair(0, i)
        project(qT_sb, wq_sb, x1_sb, 1, 64, "q1")
        ep_begin(0)
        at_tiles[1] = psum.tile([DV + 1, 1024], F32, tag="at", name="at1")
        for i in range(NT):           # h1 pairs; h0 epilogue rides PE slack
            pair(1, i)
            if i < NTH:
                ep_transpose(0, i)
        ep_stats(0)
        ep_begin(1)
        for t in range(NTH):
            ep_transpose(1, t)
        ep_stats(1)

_NC_CACHE = None


def _get_nc():
    global _NC_CACHE
    if _NC_CACHE is None:
        _NC_CACHE = build_program()
    return _NC_CACHE


def make_in_maps(x_1, x_2, Wq, Wk, Wv, bv):
    import ml_dtypes

    bf16 = ml_dtypes.bfloat16
    x1t = np.ascontiguousarray(x_1.transpose(0, 2, 1)).astype(bf16)  # [B,DM,S]
    x2t = np.ascontiguousarray(x_2.transpose(0, 2, 1)).astype(bf16)
    wkv = np.ascontiguousarray(np.concatenate([Wk, Wv], axis=1)).astype(bf16)
    wqb = Wq.astype(bf16)
    # colsum(v) + (S-1)*bv in float64 for exactness (it dominates t and must
    # not inherit device rounding)
    vsb = (
        x_2.astype(np.float64).sum(axis=1) @ Wv.astype(np.float64)
        + np.float64(S - 1) * bv.astype(np.float64)
    ).astype(np.float32)  # [B, DV]
    return [
        {"x1t": x1t[b], "x2t": x2t[b], "wq": wqb, "wkv": wkv, "vsb": vsb[b]}
        for b in range(B)
    ]


def kernel(**inputs):
    x_1 = np.asarray(inputs["x_1"], np.float32)
    x_2 = np.asarray(inputs["x_2"], np.float32)
    Wq = np.asarray(inputs["Wq"], np.float32)
    Wk = np.asarray(inputs["Wk"], np.float32)
    Wv = np.asarray(inputs["Wv"], np.float32)
    bv = np.asarray(inputs["bv"], np.float32)
    gamma = np.asarray(inputs["gamma"], np.float32)
    beta = np.asarray(inputs["beta"], np.float32)

    nc = _get_nc()
    in_maps = make_in_maps(x_1, x_2, Wq, Wk, Wv, bv)
    res = run_bass_kernel_spmd(nc, in_maps, list(range(N_CORES)))
    outs = np.stack([res.results[b]["out"] for b in range(B)], axis=0)
    return (outs * gamma + beta).astype(np.float32)


# revision 12
# speedup vs baseline: 1.0860x; 1.0053x over previous
"""CrossAttention (reverse-weight) Trainium2 kernel, v2.

Data-parallel over batch B=8 across 8 NeuronCores (one batch per core).

Math (per batch), identical to v1:
    q = x1 @ Wq, k = x2 @ Wk, v = x2 @ Wv   (bq zero; bk softmax-invariant)
    E = exp(q @ k.T / 8)   (no max-shift needed: |scores| <~ 2)
    attn = (colsum(v) - (E@v)/rowsum(E)) / (S-1)
    out = layernorm(attn) * gamma + beta, with the 1/(S-1) folded into eps.
    colsum(v) + (S-1)*bv computed host-side in float64 (vsumB).

v2 changes vs v1 (which ran ~151 us):
  - All matmul operands bf16 (halves DMA, enables FWL weight loads, keeps
    fp32 PSUM accumulate). exp output et is bf16.
  - No PE transposes at all: v-tiles and the attnT epilogue are transposed
    with the DMA xbar (16-bit SBUF->SBUF), freeing PSUM and PE time.
  - Single 8-bank PSUM pool: sc[128,1024]x2 (banks 0-3), at[65,1024]x1
    (4-5), proj[128,1024]x1 (6-7, rotating kv-h0 -> q-h0 -> kv-h1 -> q-h1
    in DMA arrival order x2h0, x1h0, x2h1, x1h1).
  - Stage 2 is h-outer (q-half outer, s-tile inner) so half 0's epilogue
    overlaps half 1's score/exp/attn pipeline; ACT exp in [128,1024]
    instructions is the expected steady-state bottleneck (~1.15 us/pair).
"""

import numpy as np

import concourse.bacc as bacc
import concourse.tile as tile
from concourse import mybir
from concourse.bass_utils import run_bass_kernel_spmd

F32 = mybir.dt.float32
BF16 = mybir.dt.bfloat16
AF = mybir.ActivationFunctionType
ALU = mybir.AluOpType

B, S, DM, DK, DV = 8, 2048, 768, 64, 64
NT = S // 128           # 16 s-tiles / q-tiles
NTH = NT // 2           # 8 tiles per q-half
NC_CHUNKS = DM // 128   # 6 contraction chunks
EPS_EFF = 1e-5 * float(S - 1) * float(S - 1)  # 41.90209
N_CORES = 8


def build_program():
    nc = bacc.Bacc(None)

    x1t = nc.declare_dram_parameter("x1t", [DM, S], BF16, isOutput=False)
    x2t = nc.declare_dram_parameter("x2t", [DM, S], BF16, isOutput=False)
    wq = nc.declare_dram_parameter("wq", [DM, DK], BF16, isOutput=False)
    wkv = nc.declare_dram_parameter("wkv", [DM, 2 * DK], BF16, isOutput=False)
    vsb = nc.declare_dram_parameter("vsb", [DV], F32, isOutput=False)
    out = nc.declare_dram_parameter("out", [S, DV], F32, isOutput=True)

    with tile.TileContext(nc) as tc:
        _emit(nc, tc, x1t, x2t, wq, wkv, vsb, out)
    nc.finalize()
    return nc


def _emit(nc, tc, x1t, x2t, wq, wkv, vsb, out):
    from contextlib import ExitStack

    ctx = ExitStack()
    with ctx:
        singles = ctx.enter_context(tc.tile_pool(name="singles", bufs=1))
        xpool = ctx.enter_context(tc.tile_pool(name="xpool", bufs=1))
        sbuf = ctx.enter_context(tc.tile_pool(name="sbuf", bufs=1))
        et_pool = ctx.enter_context(tc.tile_pool(name="et_pool", bufs=3))
        ep_pool = ctx.enter_context(tc.tile_pool(name="ep_pool", bufs=2))

        from concourse.masks import make_identity

        # ---- constants / weights ----
        ident = singles.tile([128, 128], BF16)
        make_identity(nc, ident)
        eps_sb = singles.tile([128, 1], F32)
        nc.vector.memset(eps_sb, EPS_EFF)
        wq_sb = singles.tile([128, NC_CHUNKS, DK], BF16)
        nc.sync.dma_start(out=wq_sb, in_=wq.rearrange("(c p) m -> p c m", p=128))
        wkv_sb = singles.tile([128, NC_CHUNKS, 2 * DK], BF16)
        nc.sync.dma_start(out=wkv_sb, in_=wkv.rearrange("(c p) m -> p c m", p=128))
        # vsumB = colsum(v) + (S-1)*bv, host-computed, broadcast to all rows
        vsumB = singles.tile([128, DV], F32)
        nc.sync.dma_start(out=vsumB, in_=vsb.ap().partition_broadcast(128))

        # v tiles [128, NT, 66]: cols 0:64 = v rows (s-major), col 64 = -1.0
        v_sb = sbuf.tile([128, NT, DK + 2], BF16)
        m1_sb = singles.tile([128, NT], BF16)
        nc.vector.memset(m1_sb, -1.0)
        nc.vector.tensor_copy(v_sb[:, :, DK], m1_sb)

        # ---- x piece DMAs [128, 1024] bf16; arrival order drives the proj
        # rotation on psum banks 6-7: kv-h0, q-h0, kv-h1, q-h1 ----
        x1_sb = [[None] * 2 for _ in range(NC_CHUNKS)]
        x2_sb = [[None] * 2 for _ in range(NC_CHUNKS)]

        def load_piece(dst, src, c, h, tag, eng):
            t = xpool.tile([128, 1024], BF16, tag=f"{tag}_{c}_{h}",
                           name=f"{tag}_{c}_{h}")
            eng.dma_start(
                out=t, in_=src[c * 128:(c + 1) * 128, h * 1024:(h + 1) * 1024]
            )
            dst[c][h] = t

        for c in range(NC_CHUNKS):
            load_piece(x2_sb, x2t, c, 0, "x2", nc.sync)
        for c in range(NC_CHUNKS):
            load_piece(x1_sb, x1t, c, 0, "x1", nc.sync)
        for c in range(NC_CHUNKS):
            load_piece(x2_sb, x2t, c, 1, "x2", nc.sync)
        for c in range(NC_CHUNKS):
            load_piece(x1_sb, x1t, c, 1, "x1", nc.sync)

        qT_sb = sbuf.tile([64, S], BF16)
        kv_sb = sbuf.tile([128, S], BF16)

        # ---- the single PSUM pool: 4+4+4+4 KB/partition = all 16 KB ----
        psum = ctx.enter_context(tc.tile_pool(name="psum", bufs=1, space="PSUM"))

        def project(dst_sb, w_sb, x_sb, h, nrow, tag):
            p = psum.tile([128, 1024], F32, tag="proj", name=f"proj_{tag}")
            for c in range(NC_CHUNKS):
                for blk in range(2):
                    lo = blk * 512
                    nc.tensor.matmul(
                        p[0:nrow, lo:lo + 512],
                        w_sb[:, c, :],
                        x_sb[c][h][:, lo:lo + 512],
                        start=(c == 0),
                        stop=(c == NC_CHUNKS - 1),
                    )
            nc.vector.tensor_copy(
                dst_sb[:, h * 1024:(h + 1) * 1024], p[0:nrow, :]
            )

        # ---- stage 2 + epilogue state ----
        out_sb = sbuf.tile([128, NT, DV], F32)
        mv_all = sbuf.tile([128, 2, NTH, 2], F32)   # [., half, tile, (mean,var)]
        std_all = sbuf.tile([128, 2, NTH], F32)
        rs_all = sbuf.tile([128, 2, NTH], F32)
        at_tiles = [None, None]
        ep_state = [None, None]

        def vtrans1(i):
            trp = psum.tile([128, DK], BF16, tag="proj", name=f"vtr{i}")
            nc.tensor.transpose(
                trp,
                kv_sb[64:128, i * 128:(i + 1) * 128],
                ident[64:128, 64:128],
            )
            nc.vector.tensor_copy(v_sb[:, i, 0:DK], trp)

        def pair(h, i):
            kt_i = kv_sb[0:64, i * 128:(i + 1) * 128]
            sc = psum.tile([128, 1024], F32, tag="sc", bufs=2, name="sc")
            for blk in range(2):
                qlo = h * 1024 + blk * 512
                nc.tensor.matmul(
                    sc[:, blk * 512:(blk + 1) * 512],
                    kt_i,
                    qT_sb[:, qlo:qlo + 512],
                    start=True,
                    stop=True,
                )
            et = et_pool.tile([128, 1024], BF16, tag="et", name="et")
            nc.scalar.activation(et, sc, AF.Exp, scale=0.125)
            for blk in range(2):
                nc.tensor.matmul(
                    at_tiles[h][:, blk * 512:(blk + 1) * 512],
                    v_sb[:, i, 0:DV + 1],
                    et[:, blk * 512:(blk + 1) * 512],
                    start=(i == 0),
                    stop=(i == NT - 1),
                )

        def ep_begin(h):
            # at rows: 0:64 = EV (v-major), row 64 = -rowsum. bf16 roundoff on
            # EV is ~1e-6 of the final output (EV/rowsum << layernorm scale).
            at_sbh = ep_pool.tile([DV + 1, 1024], BF16, tag="at_sb",
                                  name=f"atsb{h}")
            nc.vector.tensor_copy(at_sbh, at_tiles[h])
            a_all = ep_pool.tile([128, NTH, DV + 2], F32, tag="a_all",
                                 name=f"a_all{h}")
            ep_state[h] = (at_sbh, a_all)

        def ep_transpose(h, t):
            at_sbh, a_all = ep_state[h]
            trp = psum.tile([128, DV + 1], BF16, tag="proj",
                            name=f"eptr{h}_{t}")
            nc.tensor.transpose(
                trp,
                at_sbh[:, t * 128:(t + 1) * 128],
                ident[0:DV + 1, 0:DV + 1],
            )
            nc.vector.tensor_copy(a_all[:, t, 0:DV + 1], trp)

        def ep_stats(h):
            _, a_all = ep_state[h]
            # rneg = -1/rowsum for all 8 tiles at once (col 64 = -rowsum)
            rneg = ep_pool.tile([128, NTH], F32, tag="rneg", name=f"rneg{h}")
            nc.vector.reciprocal(rneg, a_all[:, :, DV])
            tts = []
            for t in range(NTH):
                tt = ep_pool.tile([128, DV], F32, tag=f"tt{t}", name=f"tt{t}")
                tts.append(tt)
                # t = EV * (-1/rowsum) + vsumB
                nc.vector.scalar_tensor_tensor(
                    out=tt,
                    in0=a_all[:, t, 0:DV],
                    scalar=rneg[:, t:t + 1],
                    in1=vsumB,
                    op0=ALU.mult,
                    op1=ALU.add,
                )
                stats = ep_pool.tile([128, 6], F32, tag="stats", name="stats")
                nc.vector.bn_stats(out=stats, in_=tt)
                nc.vector.bn_aggr(out=mv_all[:, h, t, :], in_=stats)
            # batched std = sqrt(var + eps) and rs = 1/std for the 8 tiles
            nc.scalar.activation(
                std_all[:, h, :], mv_all[:, h, :, 1], AF.Sqrt,
                bias=eps_sb, scale=1.0,
            )
            nc.vector.reciprocal(rs_all[:, h, :], std_all[:, h, :])
            for t in range(NTH):
                gt = t + h * NTH
                nc.vector.tensor_scalar(
                    out=out_sb[:, gt, :],
                    in0=tts[t],
                    scalar1=mv_all[:, h, t, 0:1],
                    scalar2=rs_all[:, h, t:t + 1],
                    op0=ALU.subtract,
                    op1=ALU.mult,
                )
            nc.sync.dma_start(
                out=out.rearrange("(t p) j -> p t j", p=128)[
                    :, h * NTH:(h + 1) * NTH, :
                ],
                in_=out_sb[:, h * NTH:(h + 1) * NTH, :],
            )

        # ---- interleaved emission in data-arrival order: the PE stream must
        # never make early work wait behind matmuls whose DMA lands later.
        # v-transposes ride the ACT slack inside the pair loop instead of
        # gating the first score matmul. ----
        project(kv_sb, wkv_sb, x2_sb, 0, 128, "kv0")
        project(qT_sb, wq_sb, x1_sb, 0, 64, "q0")
        at_tiles[0] = psum.tile([DV + 1, 1024], F32, tag="at", name="at0")
        for i in range(NTH):          # h0 pairs on s-tiles 0..7
            vtrans1(i)
            pair(0, i)
        project(kv_sb, wkv_sb, x2_sb, 1, 128, "kv1")
        for i in range(NTH, NT):      # h0 pairs on s-tiles 8..15
            vtrans1(i)
            pair(0, i)
        project(qT_sb, wq_sb, x1_sb, 1, 64, "q1")
        ep_begin(0)
        at_tiles[1] = psum.tile([DV + 1, 1024], F32, tag="at", name="at1")
        for i in range(NT):           # h1 pairs; h0 epilogue rides PE slack
            pair(1, i)
            if i < NTH:
                ep_transpose(0, i)
        ep_stats(0)
        ep_begin(1)
        for t in range(NTH):
            ep_transpose(1, t)
        ep_stats(1)

_NC_CACHE = None


def _get_nc():
    global _NC_CACHE
    if _NC_CACHE is None:
        _NC_CACHE = build_program()
    return _NC_CACHE


def make_in_maps(x_1, x_2, Wq, Wk, Wv, bv):
    import ml_dtypes

    bf16 = ml_dtypes.bfloat16
    x1t = np.ascontiguousarray(x_1.transpose(0, 2, 1)).astype(bf16)  # [B,DM,S]
    x2t = np.ascontiguousarray(x_2.transpose(0, 2, 1)).astype(bf16)
    wkv = np.ascontiguousarray(np.concatenate([Wk, Wv], axis=1)).astype(bf16)
    wqb = Wq.astype(bf16)
    # colsum(v) + (S-1)*bv in float64 for exactness (it dominates t and must
    # not inherit device rounding)
    vsb = (
        x_2.astype(np.float64).sum(axis=1) @ Wv.astype(np.float64)
        + np.float64(S - 1) * bv.astype(np.float64)
    ).astype(np.float32)  # [B, DV]
    return [
        {"x1t": x1t[b], "x2t": x2t[b], "wq": wqb, "wkv": wkv, "vsb": vsb[b]}
        for b in range(B)
    ]


def kernel(**inputs):
    x_1 = np.asarray(inputs["x_1"], np.float32)
    x_2 = np.asarray(inputs["x_2"], np.float32)
    Wq = np.asarray(inputs["Wq"], np.float32)
    Wk = np.asarray(inputs["Wk"], np.float32)
    Wv = np.asarray(inputs["Wv"], np.float32)
    bv = np.asarray(inputs["bv"], np.float32)
    gamma = np.asarray(inputs["gamma"], np.float32)
    beta = np.asarray(inputs["beta"], np.float32)

    nc = _get_nc()
    in_maps = make_in_maps(x_1, x_2, Wq, Wk, Wv, bv)
    res = run_bass_kernel_spmd(nc, in_maps, list(range(N_CORES)))
    outs = np.stack([res.results[b]["out"] for b in range(B)], axis=0)
    return (outs * gamma + beta).astype(np.float32)
